# revision 22
# baseline (speedup 1.0000x reference)
"""Trainium2 Bass kernel for ConvTranspose3d(32->64, k=3, s=2, p=1) + inference
BatchNorm + per-(sample,channel) spatial mean subtraction.

Math: bias / beta / running_mean cancel exactly in the mean subtraction:
    out = A_c * (convT(x) - mean_spatial(convT(x))),  A_c = gamma/sqrt(var+eps)

Decomposition: stride-2 transpose conv -> 8 output parity classes.
Per dim, output o = 2j+p: p=0 uses kernel tap k=1 (input shift s=0);
p=1 uses taps k=2 (s=0) and k=0 (s=1).  The (sh,sw) shift variants of x are
baked into 4 partition groups of one SBUF tensor T1 (128 = 4x32ci partitions);
the d shift is a free-dim offset.  Each class = 1-2 matmul passes with
K in {32,64,128}; classes (pd=0, pd=1) pair into psum col halves (M=64 each)
so the epilogue runs on full 128 partitions.

The spatial mean is computed analytically from 27 box sums of x (full /
drop-first / drop-last per dim) fed through the same weight tiles as 12 tiny
matmuls (N=1), then folded into the epilogue's per-partition scale+bias.

Sharding: data-parallel, one sample per core (B=8, 8 cores).
"""

import numpy as np

B, CIN, COUT = 8, 32, 64
D, H, W = 16, 32, 32
DO, HO, WO = 31, 63, 63
EPS = 1e-5
NSPAT = DO * HO * WO

GROUPS = [(0, 0), (0, 1), (1, 0), (1, 1)]          # g = (sh, sw)
HW = [((0, 0), 32), ((0, 1), 64), ((1, 0), 128), ((1, 1), 128)]  # ((ph,pw), K)
PDTAP = [(0, 0), (1, 0), (1, 1)]                    # (pd, sd)


def _kmap(p, s):
    return 1 if p == 0 else (2 if s == 0 else 0)


def _rmap(p, s):
    # box range per dim: Full / drop-Last / drop-fiRst
    return 0 if p == 0 else (1 if s == 0 else 2)


def _tap_groups(ph, pw):
    return [gi for gi, (sh, sw) in enumerate(GROUPS)
            if not ((ph == 0 and sh != 0) or (pw == 0 and sw != 0))]


def build_nc():
    import concourse.bacc as bacc
    import concourse.mybir as mybir
    import concourse.tile as tile

    f32 = mybir.dt.float32
    bf16 = mybir.dt.bfloat16
    Alu = mybir.AluOpType
    Act = mybir.ActivationFunctionType

    nc = bacc.Bacc()
    x_d = nc.declare_dram_parameter("x", [CIN, D, H, W], f32, isOutput=False)
    w_d = nc.declare_dram_parameter("w", [CIN, COUT, 3, 3, 3], f32, isOutput=False)
    g_d = nc.declare_dram_parameter("gamma", [COUT], f32, isOutput=False)
    v_d = nc.declare_dram_parameter("rvar", [COUT], f32, isOutput=False)
    o_d = nc.declare_dram_parameter("out", [COUT, DO, HO, WO], f32, isOutput=True)

    with tile.TileContext(nc) as tc:
        with (
            tc.tile_pool(name="singles", bufs=1) as singles,
            tc.tile_pool(name="stag", bufs=3) as stpool,
            tc.tile_pool(name="psum", bufs=8, space="PSUM") as pspool,
        ):
            # ---------------- prologue: loads ----------------
            T1 = singles.tile([128, 17, H, W], bf16)      # variant tensor
            T1f = T1[:].rearrange("p d h w -> p d (h w)")
            Ws = singles.tile([128, 12, COUT], f32)       # weight staging
            Wt = singles.tile([128, 12, COUT], bf16)      # 12 weight passes
            packx = singles.tile([128, 4, H, W], f32)     # ci*4+dgrp packing

            # x, packed for fast reductions (partition = ci*4 + dgrp)
            nc.sync.dma_start(
                out=packx[:],
                in_=x_d[:].rearrange("c (g s) h w -> (c g) s h w", g=4))
            # bf16 cast of x (same c-major packing)
            xbf = singles.tile([128, 4 * H * W], bf16)
            nc.vector.tensor_copy(out=xbf[:], in_=packx[:].rearrange(
                "p s h w -> p (s h w)"))

            # zero all of T1 once (covers the d=16 / jh=31 / jw=31 pads);
            # on vector so downstream DMAs wait on a single engine sem
            nc.vector.memset(T1[:], 0.0)
            # T1 group 0 = x as-is (d slices 0..15); flat enumerations align
            nc.sync.dma_start(
                out=T1[0:32, 0:16, :, :].rearrange("p d h w -> p (d h w)"),
                in_=xbf[:])
            # groups 1..3: shifted copies (3-dim DMA-friendly views)
            T1r = T1[:].rearrange("p d h w -> p (d h) w")   # (128, 544, 32)
            # g1 = (0,1): w shift of g0
            nc.sync.dma_start(out=T1r[32:64, 0:512, 0:31],
                              in_=T1r[0:32, 0:512, 1:32])
            # g2 = (1,0): h shift of g0, via flattened (h w)
            nc.sync.dma_start(out=T1f[64:96, 0:16, 0:992],
                              in_=T1f[0:32, 0:16, 32:1024])
            # g3 = (1,1): h shift of g1 (brings g1's zeroed w-pad col along)
            nc.sync.dma_start(out=T1f[96:128, 0:16, 0:992],
                              in_=T1f[32:64, 0:16, 32:1024])

            # weight tiles (zero-padded to the T1 group stacking)
            nc.vector.memset(Ws[:], 0.0)
            for hwi, ((ph, pw), K) in enumerate(HW):
                for ti, (pd, sd) in enumerate(PDTAP):
                    pi = hwi * 3 + ti
                    kd = _kmap(pd, sd)
                    for gi in _tap_groups(ph, pw):
                        sh, sw = GROUPS[gi]
                        nc.sync.dma_start(
                            out=Ws[32 * gi:32 * gi + 32, pi, :],
                            in_=w_d[:, :, kd, _kmap(ph, sh), _kmap(pw, sw)])
            nc.vector.tensor_copy(out=Wt[:], in_=Ws[:])

            # per-channel scale A = gamma / sqrt(var + eps), as a column
            gcol = singles.tile([COUT, 1], f32)
            vcol = singles.tile([COUT, 1], f32)
            nc.sync.dma_start(out=gcol[:], in_=g_d[:].unsqueeze(1))
            nc.sync.dma_start(out=vcol[:], in_=v_d[:].unsqueeze(1))
            eps_t = singles.tile([COUT, 1], f32)
            nc.vector.memset(eps_t[:], EPS)
            sq = singles.tile([COUT, 1], f32)
            nc.scalar.activation(out=sq[:], in_=vcol[:], func=Act.Sqrt,
                                 bias=eps_t[:], scale=1.0)
            rc = singles.tile([COUT, 1], f32)
            nc.vector.reciprocal(out=rc[:], in_=sq[:])
            acol = singles.tile([COUT, 1], f32)
            nc.vector.tensor_mul(acol[:], rc[:], gcol[:])
            arep = singles.tile([128, 1], f32)
            nc.sync.dma_start(out=arep[0:64, :], in_=acol[:])
            nc.sync.dma_start(out=arep[64:128, :], in_=acol[:])

            # ---------------- mean: 27 box sums of x ----------------
            # stage A on packed x (128 partitions)
            hsum = singles.tile([128, 4, H], f32)
            nc.vector.reduce_sum(out=hsum[:], in_=packx[:], axis=mybir.AxisListType.X)
            wc0 = singles.tile([128, 4, H], f32)
            wc31 = singles.tile([128, 4, H], f32)
            nc.gpsimd.tensor_copy(out=wc0[:], in_=packx[:, :, :, 0:1].squeeze(3))
            nc.gpsimd.tensor_copy(out=wc31[:], in_=packx[:, :, :, 31:32].squeeze(3))
            # gather to (32ci, 16d, 32h)
            rows = {}
            for name, src in (("F", hsum), ("c0", wc0), ("c31", wc31)):
                t = singles.tile([CIN, D, H], f32, name=f"rows_{name}")
                # partition packing is ci-major, so flat enumerations align:
                # src (ci,g) x (s,h)  ==  dst ci x (g,s,h)
                nc.sync.dma_start(
                    out=t[:].rearrange("c d h -> c (d h)"),
                    in_=src[:].rearrange("p s h -> p (s h)"))
                rows[name] = t
            rowL = singles.tile([CIN, D, H], f32)
            rowR = singles.tile([CIN, D, H], f32)
            nc.vector.tensor_sub(rowL[:], rows["F"][:], rows["c31"][:])
            nc.vector.tensor_sub(rowR[:], rows["F"][:], rows["c0"][:])
            rw_t = [rows["F"], rowL, rowR]

            # per rw: h-reduce + h endpoints -> dvecs (32, 16)
            dvec = {}
            for rw in range(3):
                t = rw_t[rw]
                dh = singles.tile([CIN, D], f32, name=f"dh_{rw}")
                nc.vector.reduce_sum(out=dh[:], in_=t[:], axis=mybir.AxisListType.X)
                h0 = singles.tile([CIN, D], f32, name=f"h0_{rw}")
                h31 = singles.tile([CIN, D], f32, name=f"h31_{rw}")
                nc.gpsimd.tensor_copy(out=h0[:], in_=t[:, :, 0:1].squeeze(2))
                nc.gpsimd.tensor_copy(out=h31[:], in_=t[:, :, 31:32].squeeze(2))
                dvec[(0, rw)] = dh
                dL = singles.tile([CIN, D], f32, name=f"dL_{rw}")
                dR = singles.tile([CIN, D], f32, name=f"dR_{rw}")
                nc.vector.tensor_sub(dL[:], dh[:], h31[:])
                nc.vector.tensor_sub(dR[:], dh[:], h0[:])
                dvec[(1, rw)] = dL
                dvec[(2, rw)] = dR

            # 27 final d-range reduces -> Smat (32, 27)
            smat = singles.tile([CIN, 27], f32)
            drange = [(0, D), (0, D - 1), (1, D)]
            tap_idx = {}
            ti_ = 0
            for rd in range(3):
                for rh in range(3):
                    for rw in range(3):
                        a, b = drange[rd]
                        nc.vector.reduce_sum(
                            out=smat[:, ti_:ti_ + 1],
                            in_=dvec[(rh, rw)][:, a:b],
                            axis=mybir.AxisListType.X)
                        tap_idx[(rd, rh, rw)] = ti_
                        ti_ += 1

            # scatter into the stacked S columns (128, 12), bf16 for matmul
            smat_bf = singles.tile([CIN, 27], bf16)
            nc.vector.tensor_copy(out=smat_bf[:], in_=smat[:])
            scol = singles.tile([128, 12], bf16)
            nc.vector.memset(scol[:], 0.0)
            for hwi, ((ph, pw), K) in enumerate(HW):
                for ti, (pd, sd) in enumerate(PDTAP):
                    pi = hwi * 3 + ti
                    for gi in _tap_groups(ph, pw):
                        sh, sw = GROUPS[gi]
                        t = tap_idx[(_rmap(pd, sd), _rmap(ph, sh), _rmap(pw, sw))]
                        nc.sync.dma_start(
                            out=scol[32 * gi:32 * gi + 32, pi:pi + 1],
                            in_=smat_bf[:, t:t + 1])

            # 12 tiny matmuls accumulate per-channel conv sums
            mps = pspool.tile([128, 512], f32, tag="main_ps")
            n_pass = len(HW) * len(PDTAP)
            for hwi, ((ph, pw), K) in enumerate(HW):
                for ti, (pd, sd) in enumerate(PDTAP):
                    pi = hwi * 3 + ti
                    nc.tensor.matmul(
                        mps[0:COUT, 0:1],
                        Wt[0:K, pi, :],
                        scol[0:K, pi:pi + 1],
                        start=(pi == 0), stop=(pi == n_pass - 1))
            # bias = -A * mean
            msb = singles.tile([COUT, 1], f32)
            nc.scalar.activation(out=msb[:], in_=mps[0:COUT, 0:1],
                                 func=Act.Copy, bias=0.0, scale=1.0 / NSPAT)
            bcol = singles.tile([COUT, 1], f32)
            nc.vector.tensor_scalar(out=bcol[:], in0=msb[:], scalar1=acol[:],
                                    scalar2=-1.0, op0=Alu.mult, op1=Alu.mult)
            brep = singles.tile([128, 1], f32)
            nc.sync.dma_start(out=brep[0:64, :], in_=bcol[:])
            nc.sync.dma_start(out=brep[64:128, :], in_=bcol[:])

            # ---------------- main loop ----------------
            epi = 0
            for jd in range(16):
                last = jd == 15          # only even output plane d=30
                stag = stpool.tile([128, HO, WO], f32)
                for nt in range(2):
                    for hwi, ((ph, pw), K) in enumerate(HW):
                        ps = pspool.tile([128, 512], f32, tag="main_ps")
                        psv = ps[:].rearrange("p (a b) -> p a b", a=16)
                        # col half A: pd=0 (one pass); col half B: pd=1 (two)
                        for ti, (pd, sd) in enumerate(PDTAP):
                            if last and pd == 1:
                                continue
                            pi = hwi * 3 + ti
                            nc.tensor.matmul(
                                ps[64 * pd:64 * pd + 64, :],
                                Wt[0:K, pi, :],
                                T1f[0:K, jd + sd, 512 * nt:512 * nt + 512],
                                start=(ti <= 1), stop=(ti != 1))
                        # epilogue: out = A*psum + bias, interleaved into plane
                        jhc = 16 if (ph == 0 or nt == 0) else 15
                        jwc = W - pw
                        np_ = 64 if last else 128
                        h0 = 32 * nt + ph
                        dest = stag[0:np_, h0:min(h0 + 2 * jhc, HO):2,
                                    pw:min(pw + 2 * jwc, WO):2]
                        src = psv[0:np_, 0:jhc, 0:jwc]
                        if epi % 2 == 0:
                            nc.scalar.activation(
                                out=dest, in_=src, func=Act.Identity,
                                bias=brep[0:np_, :], scale=arep[0:np_, :])
                        else:
                            nc.vector.tensor_scalar(
                                out=dest, in0=src,
                                scalar1=arep[0:np_, :], scalar2=brep[0:np_, :],
                                op0=Alu.mult, op1=Alu.add)
                        epi += 1
                # SBUF partitions are (q=d-parity, c) q-major, matching the
                # transposed DRAM enumeration (q, c, h, w)
                if last:
                    nc.sync.dma_start(
                        out=o_d[:, 30:31, :, :].transpose([1, 0, 2, 3]),
                        in_=stag[0:64, :, :])
                else:
                    nc.sync.dma_start(
                        out=o_d[:, 2 * jd:2 * jd + 2, :, :].transpose([1, 0, 2, 3]),
                        in_=stag[:])
    nc.compile()
    return nc


_NC = None


def _get_nc():
    global _NC
    if _NC is None:
        _NC = build_nc()
    return _NC


def run(inputs, trace=False):
    from concourse.bass_utils import run_bass_kernel_spmd

    nc = _get_nc()
    x = np.ascontiguousarray(np.asarray(inputs["x"], dtype=np.float32))
    w = np.ascontiguousarray(np.asarray(inputs["weight"], dtype=np.float32))
    gamma = np.ascontiguousarray(np.asarray(inputs["gamma"], dtype=np.float32))
    rvar = np.ascontiguousarray(np.asarray(inputs["running_var"], dtype=np.float32))
    in_maps = [{"x": x[k], "w": w, "gamma": gamma, "rvar": rvar} for k in range(B)]
    res = run_bass_kernel_spmd(nc, in_maps, core_ids=list(range(B)), trace=trace)
    out = np.stack([res.results[k]["out"] for k in range(B)], axis=0)
    return out, res


def kernel(**inputs) -> np.ndarray:
    out, _ = run(inputs, trace=False)
    return out


# ---------------------------------------------------------------------------
# Benchmarking helpers (test.py only; the grader uses kernel() above).
# The axon client has no NTFF hook, so we wall-clock a reusable sharded jit
# with device-resident inputs, calibrated against a null kernel.
# ---------------------------------------------------------------------------

def _build_sharded_fn(nc, n_cores=B):
    import jax
    from jax.experimental.shard_map import shard_map
    from jax.sharding import Mesh, PartitionSpec
    import concourse.mybir as mybir
    from concourse import bass2jax

    bass2jax.install_neuronx_cc_hook()
    partition_name = (nc.partition_id_tensor.name
                      if nc.partition_id_tensor else None)
    in_names, out_names, out_avals, zero_outs = [], [], [], []
    for alloc in nc.m.functions[0].allocations:
        if not isinstance(alloc, mybir.MemoryLocationSet):
            continue
        name = alloc.memorylocations[0].name
        if alloc.kind == "ExternalInput":
            if name != partition_name:
                in_names.append(name)
        elif alloc.kind == "ExternalOutput":
            out_names.append(name)
            shape = tuple(alloc.tensor_shape)
            dtype = mybir.dt.np(alloc.dtype)
            out_avals.append(jax.core.ShapedArray(shape, dtype))
            zero_outs.append(np.zeros(shape, dtype))
    n_params = len(in_names)
    all_names = in_names + out_names
    if partition_name is not None:
        all_names = all_names + [partition_name]
    donate = tuple(range(n_params, n_params + len(out_names)))

    def _body(*args):
        operands = list(args)
        if partition_name is not None:
            operands.append(bass2jax.partition_id_tensor())
        outs = bass2jax._bass_exec_p.bind(
            *operands,
            out_avals=tuple(out_avals),
            in_names=tuple(all_names),
            out_names=tuple(out_names),
            lowering_input_output_aliases=(),
            sim_require_finite=True,
            sim_require_nnan=True,
            nc=nc,
        )
        return tuple(outs)

    devices = jax.devices()[:n_cores]
    mesh = Mesh(np.asarray(devices), ("core",))
    nspec = (PartitionSpec("core"),)
    fn = jax.jit(
        shard_map(_body, mesh=mesh, in_specs=nspec * (n_params + len(out_names)),
                  out_specs=nspec * len(out_names), check_rep=False),
        donate_argnums=donate, keep_unused=True)
    return fn, mesh, in_names, out_names, out_avals, zero_outs


def _build_null_nc():
    import concourse.bacc as bacc
    import concourse.mybir as mybir
    import concourse.tile as tile

    f32 = mybir.dt.float32
    nc = bacc.Bacc()
    a = nc.declare_dram_parameter("a", [1, 32], f32, isOutput=False)
    bout = nc.declare_dram_parameter("b", [1, 32], f32, isOutput=True)
    with tile.TileContext(nc) as tc:
        with tc.tile_pool(name="p", bufs=1) as pool:
            t = pool.tile([1, 32], f32)
            nc.sync.dma_start(out=t[:], in_=a[:])
            nc.sync.dma_start(out=bout[:], in_=t[:])
    nc.compile()
    return nc


def _bench_nc(nc, per_core_inputs, iters):
    """per_core_inputs: list over cores of dict name->array. Returns
    (list of wall seconds, outputs of last iter as list over cores)."""
    import time
    import jax
    from jax.sharding import NamedSharding, PartitionSpec

    n_cores = len(per_core_inputs)
    fn, mesh, in_names, out_names, out_avals, zero_outs = _build_sharded_fn(
        nc, n_cores)
    sh = NamedSharding(mesh, PartitionSpec("core"))
    in_dev = [
        jax.device_put(
            np.concatenate([np.asarray(per_core_inputs[c][n])
                            for c in range(n_cores)], axis=0), sh)
        for n in in_names
    ]

    def fresh_zeros():
        return [jax.device_put(
            np.zeros((n_cores * z.shape[0], *z.shape[1:]), z.dtype), sh)
            for z in zero_outs]

    # warmup (compile)
    outs = fn(*in_dev, *fresh_zeros())
    jax.block_until_ready(outs)
    times = []
    for _ in range(iters):
        zs = fresh_zeros()
        jax.block_until_ready(zs)
        t0 = time.perf_counter()
        outs = fn(*in_dev, *zs)
        jax.block_until_ready(outs)
        times.append(time.perf_counter() - t0)
    res = [
        {n: np.asarray(outs[i]).reshape(n_cores, *out_avals[i].shape)[c]
         for i, n in enumerate(out_names)}
        for c in range(n_cores)
    ]
    return times, res


def bench(inputs, iters=8):
    x = np.ascontiguousarray(np.asarray(inputs["x"], dtype=np.float32))
    w = np.ascontiguousarray(np.asarray(inputs["weight"], dtype=np.float32))
    gamma = np.ascontiguousarray(np.asarray(inputs["gamma"], dtype=np.float32))
    rvar = np.ascontiguousarray(np.asarray(inputs["running_var"], dtype=np.float32))
    per_core = [{"x": x[k], "w": w, "gamma": gamma, "rvar": rvar}
                for k in range(B)]
    times, res = _bench_nc(_get_nc(), per_core, iters)
    out = np.stack([res[k]["out"] for k in range(B)], axis=0)

    null_per_core = [{"a": np.zeros((1, 32), np.float32)} for _ in range(B)]
    null_times, _ = _bench_nc(_build_null_nc(), null_per_core, iters)
    return out, times, null_times



# revision 23
# speedup vs baseline: 33.4143x; 33.4143x over previous
"""Trainium2 Bass kernel for ConvTranspose3d(32->64, k=3, s=2, p=1) + inference
BatchNorm + per-(sample,channel) spatial mean subtraction.

Math: bias / beta / running_mean cancel exactly in the mean subtraction:
    out = A_c * (convT(x) - mean_spatial(convT(x))),  A_c = gamma/sqrt(var+eps)

Decomposition: stride-2 transpose conv -> 8 output parity classes.
Per dim, output o = 2j+p: p=0 uses kernel tap k=1 (input shift s=0);
p=1 uses taps k=2 (s=0) and k=0 (s=1).  The (sh,sw) shift variants of x are
baked into 4 partition groups of one SBUF tensor T1 (128 = 4x32ci partitions);
the d shift is a free-dim offset.  Each class = 1-2 matmul passes with
K in {32,64,128}; classes (pd=0, pd=1) pair into psum col halves (M=64 each)
so the epilogue runs on full 128 partitions.

The spatial mean is computed analytically from 27 box sums of x (full /
drop-first / drop-last per dim) fed through the same weight tiles as 12 tiny
matmuls (N=1), then folded into the epilogue's per-partition scale+bias.

Sharding: data-parallel, one sample per core (B=8, 8 cores).
"""

import numpy as np

B, CIN, COUT = 8, 32, 64
D, H, W = 16, 32, 32
DO, HO, WO = 31, 63, 63
EPS = 1e-5
NSPAT = DO * HO * WO

GROUPS = [(0, 0), (0, 1), (1, 0), (1, 1)]          # g = (sh, sw)
HW = [((0, 0), 32), ((0, 1), 64), ((1, 0), 128), ((1, 1), 128)]  # ((ph,pw), K)
PDTAP = [(0, 0), (1, 0), (1, 1)]                    # (pd, sd)


def _kmap(p, s):
    return 1 if p == 0 else (2 if s == 0 else 0)


def _rmap(p, s):
    # box range per dim: Full / drop-Last / drop-fiRst
    return 0 if p == 0 else (1 if s == 0 else 2)


def _tap_groups(ph, pw):
    return [gi for gi, (sh, sw) in enumerate(GROUPS)
            if not ((ph == 0 and sh != 0) or (pw == 0 and sw != 0))]


def build_nc():
    import concourse.bacc as bacc
    import concourse.mybir as mybir
    import concourse.tile as tile

    f32 = mybir.dt.float32
    bf16 = mybir.dt.bfloat16
    Alu = mybir.AluOpType
    Act = mybir.ActivationFunctionType

    nc = bacc.Bacc()
    x_d = nc.declare_dram_parameter("x", [CIN, D, H, W], f32, isOutput=False)
    w_d = nc.declare_dram_parameter("w", [CIN, COUT, 3, 3, 3], f32, isOutput=False)
    g_d = nc.declare_dram_parameter("gamma", [COUT], f32, isOutput=False)
    v_d = nc.declare_dram_parameter("rvar", [COUT], f32, isOutput=False)
    o_d = nc.declare_dram_parameter("out", [COUT, DO, HO, WO], f32, isOutput=True)

    with tile.TileContext(nc) as tc:
        with (
            tc.tile_pool(name="singles", bufs=1) as singles,
            tc.tile_pool(name="stag", bufs=3) as stpool,
            tc.tile_pool(name="psum", bufs=8, space="PSUM") as pspool,
        ):
            # ---------------- prologue: loads ----------------
            T1 = singles.tile([128, 17, H, W], bf16)      # variant tensor
            T1f = T1[:].rearrange("p d h w -> p d (h w)")
            Ws = singles.tile([128, 12, COUT], f32)       # weight staging
            Wt = singles.tile([128, 12, COUT], bf16)      # 12 weight passes
            packx = singles.tile([128, 4, H, W], f32)     # ci*4+dgrp packing

            # x, packed for fast reductions (partition = ci*4 + dgrp)
            nc.sync.dma_start(
                out=packx[:],
                in_=x_d[:].rearrange("c (g s) h w -> (c g) s h w", g=4))
            # bf16 cast of x (same c-major packing)
            xbf = singles.tile([128, 4 * H * W], bf16)
            nc.vector.tensor_copy(out=xbf[:], in_=packx[:].rearrange(
                "p s h w -> p (s h w)"))

            # zero all of T1 once (covers the d=16 / jh=31 / jw=31 pads);
            # on vector so downstream DMAs wait on a single engine sem
            nc.vector.memset(T1[:], 0.0)
            # T1 group 0 = x as-is (d slices 0..15); flat enumerations align
            nc.sync.dma_start(
                out=T1[0:32, 0:16, :, :].rearrange("p d h w -> p (d h w)"),
                in_=xbf[:])
            # groups 1..3: shifted copies (3-dim DMA-friendly views)
            T1r = T1[:].rearrange("p d h w -> p (d h) w")   # (128, 544, 32)
            # g1 = (0,1): w shift of g0
            nc.sync.dma_start(out=T1r[32:64, 0:512, 0:31],
                              in_=T1r[0:32, 0:512, 1:32])
            # g2 = (1,0): h shift of g0, via flattened (h w)
            nc.sync.dma_start(out=T1f[64:96, 0:16, 0:992],
                              in_=T1f[0:32, 0:16, 32:1024])
            # g3 = (1,1): h shift of g1 (brings g1's zeroed w-pad col along)
            nc.sync.dma_start(out=T1f[96:128, 0:16, 0:992],
                              in_=T1f[32:64, 0:16, 32:1024])

            # weight tiles (zero-padded to the T1 group stacking)
            nc.vector.memset(Ws[:], 0.0)
            for hwi, ((ph, pw), K) in enumerate(HW):
                for ti, (pd, sd) in enumerate(PDTAP):
                    pi = hwi * 3 + ti
                    kd = _kmap(pd, sd)
                    for gi in _tap_groups(ph, pw):
                        sh, sw = GROUPS[gi]
                        nc.sync.dma_start(
                            out=Ws[32 * gi:32 * gi + 32, pi, :],
                            in_=w_d[:, :, kd, _kmap(ph, sh), _kmap(pw, sw)])
            nc.vector.tensor_copy(out=Wt[:], in_=Ws[:])

            # per-channel scale A = gamma / sqrt(var + eps), as a column
            gcol = singles.tile([COUT, 1], f32)
            vcol = singles.tile([COUT, 1], f32)
            nc.sync.dma_start(out=gcol[:], in_=g_d[:].unsqueeze(1))
            nc.sync.dma_start(out=vcol[:], in_=v_d[:].unsqueeze(1))
            eps_t = singles.tile([COUT, 1], f32)
            nc.vector.memset(eps_t[:], EPS)
            sq = singles.tile([COUT, 1], f32)
            nc.scalar.activation(out=sq[:], in_=vcol[:], func=Act.Sqrt,
                                 bias=eps_t[:], scale=1.0)
            rc = singles.tile([COUT, 1], f32)
            nc.vector.reciprocal(out=rc[:], in_=sq[:])
            acol = singles.tile([COUT, 1], f32)
            nc.vector.tensor_mul(acol[:], rc[:], gcol[:])
            arep = singles.tile([128, 1], f32)
            nc.sync.dma_start(out=arep[0:64, :], in_=acol[:])
            nc.sync.dma_start(out=arep[64:128, :], in_=acol[:])

            # ---------------- mean: 27 box sums of x ----------------
            # stage A on packed x (128 partitions)
            hsum = singles.tile([128, 4, H], f32)
            nc.vector.reduce_sum(out=hsum[:], in_=packx[:], axis=mybir.AxisListType.X)
            wc0 = singles.tile([128, 4, H], f32)
            wc31 = singles.tile([128, 4, H], f32)
            nc.gpsimd.tensor_copy(out=wc0[:], in_=packx[:, :, :, 0:1].squeeze(3))
            nc.gpsimd.tensor_copy(out=wc31[:], in_=packx[:, :, :, 31:32].squeeze(3))
            # gather to (32ci, 16d, 32h)
            rows = {}
            for name, src in (("F", hsum), ("c0", wc0), ("c31", wc31)):
                t = singles.tile([CIN, D, H], f32, name=f"rows_{name}")
                # partition packing is ci-major, so flat enumerations align:
                # src (ci,g) x (s,h)  ==  dst ci x (g,s,h)
                nc.sync.dma_start(
                    out=t[:].rearrange("c d h -> c (d h)"),
                    in_=src[:].rearrange("p s h -> p (s h)"))
                rows[name] = t
            rowL = singles.tile([CIN, D, H], f32)
            rowR = singles.tile([CIN, D, H], f32)
            nc.vector.tensor_sub(rowL[:], rows["F"][:], rows["c31"][:])
            nc.vector.tensor_sub(rowR[:], rows["F"][:], rows["c0"][:])
            rw_t = [rows["F"], rowL, rowR]

            # per rw: h-reduce + h endpoints -> dvecs (32, 16)
            dvec = {}
            for rw in range(3):
                t = rw_t[rw]
                dh = singles.tile([CIN, D], f32, name=f"dh_{rw}")
                nc.vector.reduce_sum(out=dh[:], in_=t[:], axis=mybir.AxisListType.X)
                h0 = singles.tile([CIN, D], f32, name=f"h0_{rw}")
                h31 = singles.tile([CIN, D], f32, name=f"h31_{rw}")
                nc.gpsimd.tensor_copy(out=h0[:], in_=t[:, :, 0:1].squeeze(2))
                nc.gpsimd.tensor_copy(out=h31[:], in_=t[:, :, 31:32].squeeze(2))
                dvec[(0, rw)] = dh
                dL = singles.tile([CIN, D], f32, name=f"dL_{rw}")
                dR = singles.tile([CIN, D], f32, name=f"dR_{rw}")
                nc.vector.tensor_sub(dL[:], dh[:], h31[:])
                nc.vector.tensor_sub(dR[:], dh[:], h0[:])
                dvec[(1, rw)] = dL
                dvec[(2, rw)] = dR

            # 27 final d-range reduces -> Smat (32, 27)
            smat = singles.tile([CIN, 27], f32)
            drange = [(0, D), (0, D - 1), (1, D)]
            tap_idx = {}
            ti_ = 0
            for rd in range(3):
                for rh in range(3):
                    for rw in range(3):
                        a, b = drange[rd]
                        nc.vector.reduce_sum(
                            out=smat[:, ti_:ti_ + 1],
                            in_=dvec[(rh, rw)][:, a:b],
                            axis=mybir.AxisListType.X)
                        tap_idx[(rd, rh, rw)] = ti_
                        ti_ += 1

            # scatter into the stacked S columns (128, 12), bf16 for matmul
            smat_bf = singles.tile([CIN, 27], bf16)
            nc.vector.tensor_copy(out=smat_bf[:], in_=smat[:])
            scol = singles.tile([128, 12], bf16)
            nc.vector.memset(scol[:], 0.0)
            for hwi, ((ph, pw), K) in enumerate(HW):
                for ti, (pd, sd) in enumerate(PDTAP):
                    pi = hwi * 3 + ti
                    for gi in _tap_groups(ph, pw):
                        sh, sw = GROUPS[gi]
                        t = tap_idx[(_rmap(pd, sd), _rmap(ph, sh), _rmap(pw, sw))]
                        nc.sync.dma_start(
                            out=scol[32 * gi:32 * gi + 32, pi:pi + 1],
                            in_=smat_bf[:, t:t + 1])

            # 12 tiny matmuls accumulate per-channel conv sums
            mps = pspool.tile([128, 512], f32, tag="main_ps")
            n_pass = len(HW) * len(PDTAP)
            for hwi, ((ph, pw), K) in enumerate(HW):
                for ti, (pd, sd) in enumerate(PDTAP):
                    pi = hwi * 3 + ti
                    nc.tensor.matmul(
                        mps[0:COUT, 0:1],
                        Wt[0:K, pi, :],
                        scol[0:K, pi:pi + 1],
                        start=(pi == 0), stop=(pi == n_pass - 1))
            # bias = -A * mean
            msb = singles.tile([COUT, 1], f32)
            nc.scalar.activation(out=msb[:], in_=mps[0:COUT, 0:1],
                                 func=Act.Copy, bias=0.0, scale=1.0 / NSPAT)
            bcol = singles.tile([COUT, 1], f32)
            nc.vector.tensor_scalar(out=bcol[:], in0=msb[:], scalar1=acol[:],
                                    scalar2=-1.0, op0=Alu.mult, op1=Alu.mult)
            brep = singles.tile([128, 1], f32)
            nc.sync.dma_start(out=brep[0:64, :], in_=bcol[:])
            nc.sync.dma_start(out=brep[64:128, :], in_=bcol[:])

            # ---------------- main loop ----------------
            epi = 0
            for jd in range(16):
                last = jd == 15          # only even output plane d=30
                stag = stpool.tile([128, HO, WO], f32)
                for nt in range(2):
                    for hwi, ((ph, pw), K) in enumerate(HW):
                        ps = pspool.tile([128, 512], f32, tag="main_ps")
                        psv = ps[:].rearrange("p (a b) -> p a b", a=16)
                        # col half A: pd=0 (one pass); col half B: pd=1 (two)
                        for ti, (pd, sd) in enumerate(PDTAP):
                            if last and pd == 1:
                                continue
                            pi = hwi * 3 + ti
                            nc.tensor.matmul(
                                ps[64 * pd:64 * pd + 64, :],
                                Wt[0:K, pi, :],
                                T1f[0:K, jd + sd, 512 * nt:512 * nt + 512],
                                start=(ti <= 1), stop=(ti != 1))
                        # epilogue: out = A*psum + bias, interleaved into plane
                        jhc = 16 if (ph == 0 or nt == 0) else 15
                        jwc = W - pw
                        np_ = 64 if last else 128
                        h0 = 32 * nt + ph
                        dest = stag[0:np_, h0:min(h0 + 2 * jhc, HO):2,
                                    pw:min(pw + 2 * jwc, WO):2]
                        src = psv[0:np_, 0:jhc, 0:jwc]
                        if epi % 2 == 0:
                            nc.scalar.activation(
                                out=dest, in_=src, func=Act.Identity,
                                bias=brep[0:np_, :], scale=arep[0:np_, :])
                        else:
                            nc.vector.tensor_scalar(
                                out=dest, in0=src,
                                scalar1=arep[0:np_, :], scalar2=brep[0:np_, :],
                                op0=Alu.mult, op1=Alu.add)
                        epi += 1
                # SBUF partitions are (q=d-parity, c) q-major, matching the
                # transposed DRAM enumeration (q, c, h, w)
                if last:
                    nc.sync.dma_start(
                        out=o_d[:, 30:31, :, :].transpose([1, 0, 2, 3]),
                        in_=stag[0:64, :, :])
                else:
                    nc.sync.dma_start(
                        out=o_d[:, 2 * jd:2 * jd + 2, :, :].transpose([1, 0, 2, 3]),
                        in_=stag[:])
    nc.compile()
    return nc


_NC = None


def _get_nc():
    global _NC
    if _NC is None:
        _NC = build_nc()
    return _NC


def run(inputs, trace=False):
    from concourse.bass_utils import run_bass_kernel_spmd

    nc = _get_nc()
    x = np.ascontiguousarray(np.asarray(inputs["x"], dtype=np.float32))
    w = np.ascontiguousarray(np.asarray(inputs["weight"], dtype=np.float32))
    gamma = np.ascontiguousarray(np.asarray(inputs["gamma"], dtype=np.float32))
    rvar = np.ascontiguousarray(np.asarray(inputs["running_var"], dtype=np.float32))
    in_maps = [{"x": x[k], "w": w, "gamma": gamma, "rvar": rvar} for k in range(B)]
    res = run_bass_kernel_spmd(nc, in_maps, core_ids=list(range(B)), trace=trace)
    out = np.stack([res.results[k]["out"] for k in range(B)], axis=0)
    return out, res


def kernel(**inputs) -> np.ndarray:
    out, _ = run(inputs, trace=False)
    return out


# ---------------------------------------------------------------------------
# Benchmarking helpers (test.py only; the grader uses kernel() above).
# ---------------------------------------------------------------------------

def enable_axon_profiling():
    """Register the missing antenv.axon_hooks shim so that
    run_bass_kernel_spmd(trace=True) can capture NTFF profiles through the
    axon PJRT .so (see trn_agent_boot.trn_boot)."""
    import sys
    import types
    try:
        import antenv.axon_hooks  # noqa: F401
        return True
    except ImportError:
        pass
    mod = types.ModuleType("antenv.axon_hooks")
    mod._hook = None

    def set_axon_ntff_profile_hook(h):
        mod._hook = h

    def get_axon_ntff_profile_hook():
        return mod._hook

    mod.set_axon_ntff_profile_hook = set_axon_ntff_profile_hook
    mod.get_axon_ntff_profile_hook = get_axon_ntff_profile_hook
    sys.modules["antenv.axon_hooks"] = mod
    import antenv
    antenv.axon_hooks = mod
    from trn_agent_boot.trn_boot import _ntff_profile_via_ctypes
    hook = _ntff_profile_via_ctypes('/opt/axon/libaxon_pjrt.so')
    if hook is None:
        return False
    mod._hook = hook
    return True

def _build_sharded_fn(nc, n_cores=B):
    import jax
    from jax.experimental.shard_map import shard_map
    from jax.sharding import Mesh, PartitionSpec
    import concourse.mybir as mybir
    from concourse import bass2jax

    bass2jax.install_neuronx_cc_hook()
    partition_name = (nc.partition_id_tensor.name
                      if nc.partition_id_tensor else None)
    in_names, out_names, out_avals, zero_outs = [], [], [], []
    for alloc in nc.m.functions[0].allocations:
        if not isinstance(alloc, mybir.MemoryLocationSet):
            continue
        name = alloc.memorylocations[0].name
        if alloc.kind == "ExternalInput":
            if name != partition_name:
                in_names.append(name)
        elif alloc.kind == "ExternalOutput":
            out_names.append(name)
            shape = tuple(alloc.tensor_shape)
            dtype = mybir.dt.np(alloc.dtype)
            out_avals.append(jax.core.ShapedArray(shape, dtype))
            zero_outs.append(np.zeros(shape, dtype))
    n_params = len(in_names)
    all_names = in_names + out_names
    if partition_name is not None:
        all_names = all_names + [partition_name]
    donate = tuple(range(n_params, n_params + len(out_names)))

    def _body(*args):
        operands = list(args)
        if partition_name is not None:
            operands.append(bass2jax.partition_id_tensor())
        outs = bass2jax._bass_exec_p.bind(
            *operands,
            out_avals=tuple(out_avals),
            in_names=tuple(all_names),
            out_names=tuple(out_names),
            lowering_input_output_aliases=(),
            sim_require_finite=True,
            sim_require_nnan=True,
            nc=nc,
        )
        return tuple(outs)

    devices = jax.devices()[:n_cores]
    mesh = Mesh(np.asarray(devices), ("core",))
    nspec = (PartitionSpec("core"),)
    fn = jax.jit(
        shard_map(_body, mesh=mesh, in_specs=nspec * (n_params + len(out_names)),
                  out_specs=nspec * len(out_names), check_rep=False),
        donate_argnums=donate, keep_unused=True)
    return fn, mesh, in_names, out_names, out_avals, zero_outs


def _build_null_nc():
    import concourse.bacc as bacc
    import concourse.mybir as mybir
    import concourse.tile as tile

    f32 = mybir.dt.float32
    nc = bacc.Bacc()
    a = nc.declare_dram_parameter("a", [1, 32], f32, isOutput=False)
    bout = nc.declare_dram_parameter("b", [1, 32], f32, isOutput=True)
    with tile.TileContext(nc) as tc:
        with tc.tile_pool(name="p", bufs=1) as pool:
            t = pool.tile([1, 32], f32)
            nc.sync.dma_start(out=t[:], in_=a[:])
            nc.sync.dma_start(out=bout[:], in_=t[:])
    nc.compile()
    return nc


def _bench_nc(nc, per_core_inputs, iters):
    """per_core_inputs: list over cores of dict name->array. Returns
    (list of wall seconds, outputs of last iter as list over cores)."""
    import time
    import jax
    from jax.sharding import NamedSharding, PartitionSpec

    n_cores = len(per_core_inputs)
    fn, mesh, in_names, out_names, out_avals, zero_outs = _build_sharded_fn(
        nc, n_cores)
    sh = NamedSharding(mesh, PartitionSpec("core"))
    in_dev = [
        jax.device_put(
            np.concatenate([np.asarray(per_core_inputs[c][n])
                            for c in range(n_cores)], axis=0), sh)
        for n in in_names
    ]

    def fresh_zeros():
        return [jax.device_put(
            np.zeros((n_cores * z.shape[0], *z.shape[1:]), z.dtype), sh)
            for z in zero_outs]

    # warmup (compile)
    outs = fn(*in_dev, *fresh_zeros())
    jax.block_until_ready(outs)
    times = []
    for _ in range(iters):
        zs = fresh_zeros()
        jax.block_until_ready(zs)
        t0 = time.perf_counter()
        outs = fn(*in_dev, *zs)
        jax.block_until_ready(outs)
        times.append(time.perf_counter() - t0)
    res = [
        {n: np.asarray(outs[i]).reshape(n_cores, *out_avals[i].shape)[c]
         for i, n in enumerate(out_names)}
        for c in range(n_cores)
    ]
    return times, res


def bench(inputs, iters=8):
    x = np.ascontiguousarray(np.asarray(inputs["x"], dtype=np.float32))
    w = np.ascontiguousarray(np.asarray(inputs["weight"], dtype=np.float32))
    gamma = np.ascontiguousarray(np.asarray(inputs["gamma"], dtype=np.float32))
    rvar = np.ascontiguousarray(np.asarray(inputs["running_var"], dtype=np.float32))
    per_core = [{"x": x[k], "w": w, "gamma": gamma, "rvar": rvar}
                for k in range(B)]
    times, res = _bench_nc(_get_nc(), per_core, iters)
    out = np.stack([res[k]["out"] for k in range(B)], axis=0)

    null_per_core = [{"a": np.zeros((1, 32), np.float32)} for _ in range(B)]
    null_times, _ = _bench_nc(_build_null_nc(), null_per_core, iters)
    return out, times, null_times



# revision 27
# speedup vs baseline: 109.0397x; 3.2633x over previous
"""Trainium2 Bass kernel for ConvTranspose3d(32->64, k=3, s=2, p=1) + inference
BatchNorm + per-(sample,channel) spatial mean subtraction.

Math: bias / beta / running_mean cancel exactly in the mean subtraction:
    out = A_c * (convT(x) - mean_spatial(convT(x))),  A_c = gamma/sqrt(var+eps)

Decomposition: stride-2 transpose conv -> 8 output parity classes.
Per dim, output o = 2j+p: p=0 uses kernel tap k=1 (input shift s=0);
p=1 uses taps k=2 (s=0) and k=0 (s=1).  The (sh,sw) shift variants of x are
baked into 4 partition groups of one SBUF tensor T1 (128 = 4x32ci partitions);
the d shift is a free-dim offset.  Each class = 1-2 matmul passes with
K in {32,64,128}; classes (pd=0, pd=1) pair into psum col halves (M=64 each)
so the epilogue runs on full 128 partitions.

The spatial mean is computed analytically from 27 box sums of x (full /
drop-first / drop-last per dim) fed through the same weight tiles as 12 tiny
matmuls (N=1), then folded into the epilogue's per-partition scale+bias.

Sharding: data-parallel, one sample per core (B=8, 8 cores).
"""

import numpy as np

B, CIN, COUT = 8, 32, 64
D, H, W = 16, 32, 32
DO, HO, WO = 31, 63, 63
EPS = 1e-5
NSPAT = DO * HO * WO

GROUPS = [(0, 0), (0, 1), (1, 0), (1, 1)]          # g = (sh, sw)
HW = [((0, 0), 32), ((0, 1), 64), ((1, 0), 128), ((1, 1), 128)]  # ((ph,pw), K)
PDTAP = [(0, 0), (1, 0), (1, 1)]                    # (pd, sd)


def _kmap(p, s):
    return 1 if p == 0 else (2 if s == 0 else 0)


def _rmap(p, s):
    # box range per dim: Full / drop-Last / drop-fiRst
    return 0 if p == 0 else (1 if s == 0 else 2)


def _tap_groups(ph, pw):
    return [gi for gi, (sh, sw) in enumerate(GROUPS)
            if not ((ph == 0 and sh != 0) or (pw == 0 and sw != 0))]


def build_nc():
    import concourse.bacc as bacc
    import concourse.mybir as mybir
    import concourse.tile as tile

    f32 = mybir.dt.float32
    bf16 = mybir.dt.bfloat16
    Alu = mybir.AluOpType
    Act = mybir.ActivationFunctionType

    nc = bacc.Bacc()
    x_d = nc.declare_dram_parameter("x", [CIN, D, H, W], f32, isOutput=False)
    w_d = nc.declare_dram_parameter("w", [27, CIN, COUT], f32, isOutput=False)
    g_d = nc.declare_dram_parameter("gamma", [COUT], f32, isOutput=False)
    v_d = nc.declare_dram_parameter("rvar", [COUT], f32, isOutput=False)
    o_d = nc.declare_dram_parameter("out", [COUT, DO, HO, WO], f32, isOutput=True)

    with tile.TileContext(nc) as tc:
        with (
            tc.tile_pool(name="singles", bufs=1) as singles,
            tc.tile_pool(name="stag", bufs=3) as stpool,
            tc.tile_pool(name="psum", bufs=8, space="PSUM") as pspool,
        ):
            # ---------------- prologue: loads ----------------
            # T1 rows padded to 33 cols / 33 rows so the shifted group
            # copies are single flat contiguous DMAs.  Values read through
            # the pads only ever feed grid positions that the epilogue
            # trims, but we zero-init anyway to keep them finite.
            T1 = singles.tile([128, 17, 33, 33], bf16)    # variant tensor
            NT1 = 17 * 33 * 33
            T1flat = T1[:].rearrange("p d h w -> p (d h w)")
            Wt = singles.tile([128, 12, COUT], bf16)      # 12 weight passes
            packx = singles.tile([128, 4, H, W], f32)     # ci*4+dgrp packing

            # x, packed for fast reductions (partition = ci*4 + dgrp)
            nc.sync.dma_start(
                out=packx[:],
                in_=x_d[:].rearrange("c (g s) h w -> (c g) s h w", g=4))
            # bf16 cast of x into the padded-33 layout
            xbf = singles.tile([128, 4, 33, 33], bf16)
            nc.vector.memset(xbf[:], 0.0)
            nc.vector.tensor_copy(out=xbf[:, :, 0:H, 0:W], in_=packx[:])
            nc.vector.memset(T1[:], 0.0)
            # T1 group 0 = x (d 0..15): flat enumerations align (c-major)
            nc.sync.dma_start(
                out=T1flat[0:32, 0:16 * 1089],
                in_=xbf[:].rearrange("p s h w -> p (s h w)"))
            # groups 1..3: flat shifted copies (contiguous, one packet/part)
            nc.gpsimd.dma_start(out=T1flat[32:64, 0:NT1 - 1],
                                in_=T1flat[0:32, 1:NT1])        # g1 = w+1
            nc.scalar.dma_start(out=T1flat[64:96, 0:NT1 - 33],
                                in_=T1flat[0:32, 33:NT1])       # g2 = h+1
            nc.sync.dma_start(out=T1flat[96:128, 0:NT1 - 33],
                                in_=T1flat[32:64, 33:NT1])      # g3 = g1,h+1

            # weights arrive pre-transposed as (27 taps, ci, co); one
            # contiguous load, then per-tap on-chip scatters
            Wst = singles.tile([27, CIN * COUT], f32)
            nc.sync.dma_start(out=Wst[:], in_=w_d[:].rearrange("k c o -> k (c o)"))
            Wsb = singles.tile([27, CIN * COUT], bf16)
            nc.vector.tensor_copy(out=Wsb[:], in_=Wst[:])
            nc.vector.memset(Wt[:], 0.0)
            for hwi, ((ph, pw), K) in enumerate(HW):
                for ti, (pd, sd) in enumerate(PDTAP):
                    pi = hwi * 3 + ti
                    kd = _kmap(pd, sd)
                    for gi in _tap_groups(ph, pw):
                        sh, sw = GROUPS[gi]
                        kt = kd * 9 + _kmap(ph, sh) * 3 + _kmap(pw, sw)
                        # src free is (ci, co) c-major, matching dest
                        # partition x free enumeration
                        nc.gpsimd.dma_start(
                            out=Wt[32 * gi:32 * gi + 32, pi, :],
                            in_=Wsb[kt:kt + 1, :])

            # per-channel scale A = gamma / sqrt(var + eps), as a column
            gcol = singles.tile([COUT, 1], f32)
            vcol = singles.tile([COUT, 1], f32)
            nc.sync.dma_start(out=gcol[:], in_=g_d[:].unsqueeze(1))
            nc.sync.dma_start(out=vcol[:], in_=v_d[:].unsqueeze(1))
            eps_t = singles.tile([COUT, 1], f32)
            nc.vector.memset(eps_t[:], EPS)
            sq = singles.tile([COUT, 1], f32)
            nc.scalar.activation(out=sq[:], in_=vcol[:], func=Act.Sqrt,
                                 bias=eps_t[:], scale=1.0)
            rc = singles.tile([COUT, 1], f32)
            nc.vector.reciprocal(out=rc[:], in_=sq[:])
            acol = singles.tile([COUT, 1], f32)
            nc.vector.tensor_mul(acol[:], rc[:], gcol[:])
            arep = singles.tile([128, 1], f32)
            nc.sync.dma_start(out=arep[0:64, :], in_=acol[:])
            nc.sync.dma_start(out=arep[64:128, :], in_=acol[:])

            # ---------------- mean: 27 box sums of x ----------------
            # stage A on packed x (128 partitions)
            hsum = singles.tile([128, 4, H], f32)
            nc.vector.reduce_sum(out=hsum[:], in_=packx[:], axis=mybir.AxisListType.X)
            wc0 = singles.tile([128, 4, H], f32)
            wc31 = singles.tile([128, 4, H], f32)
            nc.gpsimd.tensor_copy(out=wc0[:], in_=packx[:, :, :, 0:1].squeeze(3))
            nc.gpsimd.tensor_copy(out=wc31[:], in_=packx[:, :, :, 31:32].squeeze(3))
            # gather to (32ci, 16d, 32h)
            rows = {}
            for name, src in (("F", hsum), ("c0", wc0), ("c31", wc31)):
                t = singles.tile([CIN, D, H], f32, name=f"rows_{name}")
                # partition packing is ci-major, so flat enumerations align:
                # src (ci,g) x (s,h)  ==  dst ci x (g,s,h)
                nc.sync.dma_start(
                    out=t[:].rearrange("c d h -> c (d h)"),
                    in_=src[:].rearrange("p s h -> p (s h)"))
                rows[name] = t
            rowL = singles.tile([CIN, D, H], f32)
            rowR = singles.tile([CIN, D, H], f32)
            nc.vector.tensor_sub(rowL[:], rows["F"][:], rows["c31"][:])
            nc.vector.tensor_sub(rowR[:], rows["F"][:], rows["c0"][:])
            rw_t = [rows["F"], rowL, rowR]

            # per rw: h-reduce + h endpoints -> dvecs (32, 16)
            dvec = {}
            for rw in range(3):
                t = rw_t[rw]
                dh = singles.tile([CIN, D], f32, name=f"dh_{rw}")
                nc.vector.reduce_sum(out=dh[:], in_=t[:], axis=mybir.AxisListType.X)
                h0 = singles.tile([CIN, D], f32, name=f"h0_{rw}")
                h31 = singles.tile([CIN, D], f32, name=f"h31_{rw}")
                nc.gpsimd.tensor_copy(out=h0[:], in_=t[:, :, 0:1].squeeze(2))
                nc.gpsimd.tensor_copy(out=h31[:], in_=t[:, :, 31:32].squeeze(2))
                dvec[(0, rw)] = dh
                dL = singles.tile([CIN, D], f32, name=f"dL_{rw}")
                dR = singles.tile([CIN, D], f32, name=f"dR_{rw}")
                nc.vector.tensor_sub(dL[:], dh[:], h31[:])
                nc.vector.tensor_sub(dR[:], dh[:], h0[:])
                dvec[(1, rw)] = dL
                dvec[(2, rw)] = dR

            # 27 final d-range reduces -> Smat (32, 27)
            smat = singles.tile([CIN, 27], f32)
            drange = [(0, D), (0, D - 1), (1, D)]
            tap_idx = {}
            ti_ = 0
            for rd in range(3):
                for rh in range(3):
                    for rw in range(3):
                        a, b = drange[rd]
                        nc.vector.reduce_sum(
                            out=smat[:, ti_:ti_ + 1],
                            in_=dvec[(rh, rw)][:, a:b],
                            axis=mybir.AxisListType.X)
                        tap_idx[(rd, rh, rw)] = ti_
                        ti_ += 1

            # scatter into the stacked S columns (128, 12), bf16 for matmul
            smat_bf = singles.tile([CIN, 27], bf16)
            nc.vector.tensor_copy(out=smat_bf[:], in_=smat[:])
            scol = singles.tile([128, 12], bf16)
            nc.vector.memset(scol[:], 0.0)
            for hwi, ((ph, pw), K) in enumerate(HW):
                for ti, (pd, sd) in enumerate(PDTAP):
                    pi = hwi * 3 + ti
                    for gi in _tap_groups(ph, pw):
                        sh, sw = GROUPS[gi]
                        t = tap_idx[(_rmap(pd, sd), _rmap(ph, sh), _rmap(pw, sw))]
                        nc.sync.dma_start(
                            out=scol[32 * gi:32 * gi + 32, pi:pi + 1],
                            in_=smat_bf[:, t:t + 1])

            # 12 tiny matmuls accumulate per-channel conv sums
            mps = pspool.tile([128, 512], f32, tag="main_ps")
            n_pass = len(HW) * len(PDTAP)
            for hwi, ((ph, pw), K) in enumerate(HW):
                for ti, (pd, sd) in enumerate(PDTAP):
                    pi = hwi * 3 + ti
                    nc.tensor.matmul(
                        mps[0:COUT, 0:1],
                        Wt[0:K, pi, :],
                        scol[0:K, pi:pi + 1],
                        start=(pi == 0), stop=(pi == n_pass - 1))
            # bias = -A * mean
            msb = singles.tile([COUT, 1], f32)
            nc.scalar.activation(out=msb[:], in_=mps[0:COUT, 0:1],
                                 func=Act.Copy, bias=0.0, scale=1.0 / NSPAT)
            bcol = singles.tile([COUT, 1], f32)
            nc.vector.tensor_scalar(out=bcol[:], in0=msb[:], scalar1=acol[:],
                                    scalar2=-1.0, op0=Alu.mult, op1=Alu.mult)
            brep = singles.tile([128, 1], f32)
            nc.sync.dma_start(out=brep[0:64, :], in_=bcol[:])
            nc.sync.dma_start(out=brep[64:128, :], in_=bcol[:])

            # ---------------- main loop ----------------
            epi = 0
            out_engs = [nc.sync, nc.gpsimd, nc.scalar]
            oeng_i = 0
            for jd in range(16):
                last = jd == 15          # only even output plane d=30
                stag = stpool.tile([128, HO, WO], f32)
                for nt in range(2):
                    for hwi, ((ph, pw), K) in enumerate(HW):
                        ps = pspool.tile([128, 512], f32, tag="main_ps")
                        psv = ps[:].rearrange("p (a b) -> p a b", a=16)
                        # col half A: pd=0 (one pass); col half B: pd=1 (two)
                        for ti, (pd, sd) in enumerate(PDTAP):
                            if last and pd == 1:
                                continue
                            pi = hwi * 3 + ti
                            nc.tensor.matmul(
                                ps[64 * pd:64 * pd + 64, :],
                                Wt[0:K, pi, :],
                                T1[0:K, jd + sd, 16 * nt:16 * nt + 16, 0:32],
                                start=(ti <= 1), stop=(ti != 1))
                        # epilogue: out = A*psum + bias, interleaved into plane
                        jhc = 16 if (ph == 0 or nt == 0) else 15
                        jwc = W - pw
                        np_ = 64 if last else 128
                        h0 = 32 * nt + ph
                        dest = stag[0:np_, h0:min(h0 + 2 * jhc, HO):2,
                                    pw:min(pw + 2 * jwc, WO):2]
                        src = psv[0:np_, 0:jhc, 0:jwc]
                        if epi % 2 == 0:
                            nc.scalar.activation(
                                out=dest, in_=src, func=Act.Identity,
                                bias=brep[0:np_, :], scale=arep[0:np_, :])
                        else:
                            nc.vector.tensor_scalar(
                                out=dest, in0=src,
                                scalar1=arep[0:np_, :], scalar2=brep[0:np_, :],
                                op0=Alu.mult, op1=Alu.add)
                        epi += 1
                # SBUF partitions are (q=d-parity, c); one DMA per output
                # plane, round-robin over engine queues for parallel DGE
                for q in range(1 if last else 2):
                    eng = out_engs[oeng_i % len(out_engs)]
                    oeng_i += 1
                    eng.dma_start(
                        out=o_d[:, 2 * jd + q, :, :],
                        in_=stag[64 * q:64 * q + 64, :, :])
    nc.compile()
    return nc


_NC = None


def _get_nc():
    global _NC
    if _NC is None:
        _NC = build_nc()
    return _NC


def run(inputs, trace=False):
    from concourse.bass_utils import run_bass_kernel_spmd

    nc = _get_nc()
    x = np.ascontiguousarray(np.asarray(inputs["x"], dtype=np.float32))
    w = np.asarray(inputs["weight"], dtype=np.float32)
    # (ci, co, kd, kh, kw) -> (27 taps, ci, co) so the device load is contiguous
    w = np.ascontiguousarray(w.transpose(2, 3, 4, 0, 1).reshape(27, CIN, COUT))
    gamma = np.ascontiguousarray(np.asarray(inputs["gamma"], dtype=np.float32))
    rvar = np.ascontiguousarray(np.asarray(inputs["running_var"], dtype=np.float32))
    in_maps = [{"x": x[k], "w": w, "gamma": gamma, "rvar": rvar} for k in range(B)]
    res = run_bass_kernel_spmd(nc, in_maps, core_ids=list(range(B)), trace=trace)
    out = np.stack([res.results[k]["out"] for k in range(B)], axis=0)
    return out, res


def kernel(**inputs) -> np.ndarray:
    out, _ = run(inputs, trace=False)
    return out


# ---------------------------------------------------------------------------
# Benchmarking helpers (test.py only; the grader uses kernel() above).
# ---------------------------------------------------------------------------

def enable_axon_profiling():
    """Register the missing antenv.axon_hooks shim so that
    run_bass_kernel_spmd(trace=True) can capture NTFF profiles through the
    axon PJRT .so (see trn_agent_boot.trn_boot)."""
    import sys
    import types
    try:
        import antenv.axon_hooks  # noqa: F401
        return True
    except ImportError:
        pass
    mod = types.ModuleType("antenv.axon_hooks")
    mod._hook = None

    def set_axon_ntff_profile_hook(h):
        mod._hook = h

    def get_axon_ntff_profile_hook():
        return mod._hook

    mod.set_axon_ntff_profile_hook = set_axon_ntff_profile_hook
    mod.get_axon_ntff_profile_hook = get_axon_ntff_profile_hook
    sys.modules["antenv.axon_hooks"] = mod
    import antenv
    antenv.axon_hooks = mod
    from trn_agent_boot.trn_boot import _ntff_profile_via_ctypes
    hook = _ntff_profile_via_ctypes('/opt/axon/libaxon_pjrt.so')
    if hook is None:
        return False
    mod._hook = hook
    return True

def _build_sharded_fn(nc, n_cores=B):
    import jax
    from jax.experimental.shard_map import shard_map
    from jax.sharding import Mesh, PartitionSpec
    import concourse.mybir as mybir
    from concourse import bass2jax

    bass2jax.install_neuronx_cc_hook()
    partition_name = (nc.partition_id_tensor.name
                      if nc.partition_id_tensor else None)
    in_names, out_names, out_avals, zero_outs = [], [], [], []
    for alloc in nc.m.functions[0].allocations:
        if not isinstance(alloc, mybir.MemoryLocationSet):
            continue
        name = alloc.memorylocations[0].name
        if alloc.kind == "ExternalInput":
            if name != partition_name:
                in_names.append(name)
        elif alloc.kind == "ExternalOutput":
            out_names.append(name)
            shape = tuple(alloc.tensor_shape)
            dtype = mybir.dt.np(alloc.dtype)
            out_avals.append(jax.core.ShapedArray(shape, dtype))
            zero_outs.append(np.zeros(shape, dtype))
    n_params = len(in_names)
    all_names = in_names + out_names
    if partition_name is not None:
        all_names = all_names + [partition_name]
    donate = tuple(range(n_params, n_params + len(out_names)))

    def _body(*args):
        operands = list(args)
        if partition_name is not None:
            operands.append(bass2jax.partition_id_tensor())
        outs = bass2jax._bass_exec_p.bind(
            *operands,
            out_avals=tuple(out_avals),
            in_names=tuple(all_names),
            out_names=tuple(out_names),
            lowering_input_output_aliases=(),
            sim_require_finite=True,
            sim_require_nnan=True,
            nc=nc,
        )
        return tuple(outs)

    devices = jax.devices()[:n_cores]
    mesh = Mesh(np.asarray(devices), ("core",))
    nspec = (PartitionSpec("core"),)
    fn = jax.jit(
        shard_map(_body, mesh=mesh, in_specs=nspec * (n_params + len(out_names)),
                  out_specs=nspec * len(out_names), check_rep=False),
        donate_argnums=donate, keep_unused=True)
    return fn, mesh, in_names, out_names, out_avals, zero_outs


def _build_null_nc():
    import concourse.bacc as bacc
    import concourse.mybir as mybir
    import concourse.tile as tile

    f32 = mybir.dt.float32
    nc = bacc.Bacc()
    a = nc.declare_dram_parameter("a", [1, 32], f32, isOutput=False)
    bout = nc.declare_dram_parameter("b", [1, 32], f32, isOutput=True)
    with tile.TileContext(nc) as tc:
        with tc.tile_pool(name="p", bufs=1) as pool:
            t = pool.tile([1, 32], f32)
            nc.sync.dma_start(out=t[:], in_=a[:])
            nc.sync.dma_start(out=bout[:], in_=t[:])
    nc.compile()
    return nc


def _bench_nc(nc, per_core_inputs, iters):
    """per_core_inputs: list over cores of dict name->array. Returns
    (list of wall seconds, outputs of last iter as list over cores)."""
    import time
    import jax
    from jax.sharding import NamedSharding, PartitionSpec

    n_cores = len(per_core_inputs)
    fn, mesh, in_names, out_names, out_avals, zero_outs = _build_sharded_fn(
        nc, n_cores)
    sh = NamedSharding(mesh, PartitionSpec("core"))
    in_dev = [
        jax.device_put(
            np.concatenate([np.asarray(per_core_inputs[c][n])
                            for c in range(n_cores)], axis=0), sh)
        for n in in_names
    ]

    def fresh_zeros():
        return [jax.device_put(
            np.zeros((n_cores * z.shape[0], *z.shape[1:]), z.dtype), sh)
            for z in zero_outs]

    # warmup (compile)
    outs = fn(*in_dev, *fresh_zeros())
    jax.block_until_ready(outs)
    times = []
    for _ in range(iters):
        zs = fresh_zeros()
        jax.block_until_ready(zs)
        t0 = time.perf_counter()
        outs = fn(*in_dev, *zs)
        jax.block_until_ready(outs)
        times.append(time.perf_counter() - t0)
    res = [
        {n: np.asarray(outs[i]).reshape(n_cores, *out_avals[i].shape)[c]
         for i, n in enumerate(out_names)}
        for c in range(n_cores)
    ]
    return times, res


def bench(inputs, iters=8):
    x = np.ascontiguousarray(np.asarray(inputs["x"], dtype=np.float32))
    w = np.asarray(inputs["weight"], dtype=np.float32)
    w = np.ascontiguousarray(w.transpose(2, 3, 4, 0, 1).reshape(27, CIN, COUT))
    gamma = np.ascontiguousarray(np.asarray(inputs["gamma"], dtype=np.float32))
    rvar = np.ascontiguousarray(np.asarray(inputs["running_var"], dtype=np.float32))
    per_core = [{"x": x[k], "w": w, "gamma": gamma, "rvar": rvar}
                for k in range(B)]
    times, res = _bench_nc(_get_nc(), per_core, iters)
    out = np.stack([res[k]["out"] for k in range(B)], axis=0)

    null_per_core = [{"a": np.zeros((1, 32), np.float32)} for _ in range(B)]
    null_times, _ = _bench_nc(_build_null_nc(), null_per_core, iters)
    return out, times, null_times

